# revision 10
# baseline (speedup 1.0000x reference)
"""Multi-head attention (B=4, S=2048, D=1024, H=16) on 8 TRN2 NeuronCores.

Sharding: core c = 2*b + g handles batch b (of 4) and head-group g (of 2,
8 heads / 512 model dims each).  Per core:
  - QKV projections for its batch restricted to its 512 output dims;
    qhT/khT [512, 2048] (fp8e3m4, scaled) and vh [2048, 520] (bf16) stay
    resident in SBUF.  Q/K x-inputs and weights arrive as scaled fp8e3m4
    (x*2, w*64; the 4x on scores is folded into the exp scale).
  - attention for its 8 heads in transposed-scores layout (scoresT[k, q]):
    softmax denominator via a ones-column appended to V; no max subtraction
    (scores are ~N(0, 0.08^2) after the 1/32 scale, exp cannot overflow)
  - structured to keep ScalarE (exp) saturated: per (head-pair, 512-wide
    q chunk) the two heads' scores land in one [128, 1024] PSUM tile
    (different banks, written by two concurrent 64x128 row-tiled matmuls)
    so one ACT instruction covers both heads; sc double-buffered.
  - ALL projection work (V/K/Q of the NEXT iteration, output projection of
    the PREVIOUS qc / iteration) is emitted as cost-budgeted "filler"
    chunks inside the attention kb loop, so ScalarE never idles across
    iteration boundaries: iteration r+1's V proj fillers run during r's
    qc 1-2, its K/Q pair-0 during r's qc3, and r's last outproj chunks
    during r+1's qc0.
  - output projection partial over its 512 model dims; partials
    ReduceScatter'd pairwise in 8 chunks so the collective overlaps compute
Host: pre-transposes inputs/weights, feeds per-core shards, and
reassembles the full [4, 2048, 1024] fp32 output from the 8 per-core
outputs (chunked-RS row interleaving: core 2b+g holds rows
256*ch + [128*g, 128*(g+1)) of batch b for ch in 0..7).
"""

from collections import OrderedDict

import numpy as np
import ml_dtypes

import concourse.bass as bass
import concourse.mybir as mybir
import concourse.tile as tile
from concourse import bacc
from concourse.bass_utils import run_bass_kernel_spmd

N_CORES = 8
S = 2048          # sequence length
D = 1024          # d_model
DL = 512          # local model dims (8 heads x 64)
NH = 8            # local heads
DH = 64           # head dim
W = 512           # q-chunk width for attention
NQC = S // W      # 4 q chunks
SCALE = 1.0 / 32.0  # 1/sqrt(d_model)

F32 = mybir.dt.float32
BF16 = mybir.dt.bfloat16
F8E3 = mybir.dt.float8e3

COST_PROJ = 860.0   # ns: 4-matmul proj/V half-chunk
COST_OUT = 430.0    # ns: 2-matmul outproj half-chunk

_NC_CACHE = None


def _build_nc(repeat=1, phases="abc", collective=True, overlap_c=True,
              filler_ns=460.0):
    nc = bacc.Bacc("TRN2", target_bir_lowering=False, debug=False,
                   num_devices=N_CORES)

    xq = nc.dram_tensor("xq", [D, S], F8E3, kind="ExternalInput")
    xk = nc.dram_tensor("xk", [D, S], F8E3, kind="ExternalInput")
    xv = nc.dram_tensor("xv", [D, S], BF16, kind="ExternalInput")
    wqt = nc.dram_tensor("wqt", [D, DL], F8E3, kind="ExternalInput")
    wkt = nc.dram_tensor("wkt", [D, DL], F8E3, kind="ExternalInput")
    wvt = nc.dram_tensor("wvt", [D, DL], BF16, kind="ExternalInput")
    wot = nc.dram_tensor("wot", [DL, D], BF16, kind="ExternalInput")
    y = nc.dram_tensor("y", [S // 2, D], F32, kind="ExternalOutput")

    ypart = nc.dram_tensor("ypart", [S, D], F32)
    yrs = nc.dram_tensor("yrs", [S // 2, D], F32)

    with tile.TileContext(nc) as tc:
        with (
            tc.tile_pool(name="xp", bufs=9) as xp,           # x input chunks
            tc.tile_pool(name="kqa", bufs=12) as kqa,        # khT/qhT/attn
            tc.tile_pool(name="wp", bufs=1) as wpool,        # wq/wk/wv
            tc.tile_pool(name="wop", bufs=2) as wopool,      # woT
            tc.tile_pool(name="vhp", bufs=32) as vhp,        # vh | ones
            tc.tile_pool(name="expp", bufs=4) as expp,       # exp(scores)
            tc.tile_pool(name="pvsp", bufs=2) as pvsp,       # pv psum drain
            tc.tile_pool(name="rcp", bufs=2) as rcp,         # reciprocal row
            tc.tile_pool(name="rbp", bufs=2) as rbp,         # bcast recip
            tc.tile_pool(name="stgp", bufs=2) as stgp,       # psum->dram stg
            tc.tile_pool(name="scp", bufs=2, space="PSUM") as scp,   # 4 bank
            tc.tile_pool(name="pvp", bufs=2, space="PSUM") as pvp,   # 2 bank
            tc.tile_pool(name="prp", bufs=2, space="PSUM") as prp,   # 2 bank
        ):
            fillers = OrderedDict()   # key -> (cost_ns, fn, args), FIFO
            states = {}               # rep -> tiles dict
            proj_accs = {}            # (rep, name, a[, b]) -> psum acc
            budget = [0.0]

            # ---------------- input emitters (DMA only) ----------------
            def emit_v_inputs(r):
                st = states.setdefault(r, {})
                wv_sb = wpool.tile([128, 8, DL], BF16, tag="w",
                                   name=f"r{r}_w_v")
                nc.sync.dma_start(
                    out=wv_sb[:],
                    in_=wvt[:].rearrange("(kc p) m -> p kc m", p=128))
                xv_l = []
                for kc in range(8):
                    xt = xp.tile([128, S], BF16, tag="x", bufs=9,
                                 name=f"r{r}_xv_{kc}")
                    nc.sync.dma_start(out=xt[:],
                                      in_=xv[kc * 128:(kc + 1) * 128, :])
                    xv_l.append(xt)
                st["wv"], st["xv"] = wv_sb, xv_l
                st["vh"] = [None] * 16

            def emit_k_inputs(r):
                st = states.setdefault(r, {})
                wk_sb = wpool.tile([128, 8, DL], F8E3, tag="w8", bufs=3,
                                   name=f"r{r}_w_k")
                nc.sync.dma_start(
                    out=wk_sb[:],
                    in_=wkt[:].rearrange("(kc p) m -> p kc m", p=128))
                xk_l = []
                for kc in range(8):
                    xt = xp.tile([128, S], F8E3, tag="x8", bufs=16,
                                 name=f"r{r}_xk_{kc}")
                    nc.sync.dma_start(out=xt[:],
                                      in_=xk[kc * 128:(kc + 1) * 128, :])
                    xk_l.append(xt)
                st["wk"], st["xk"] = wk_sb, xk_l

            def emit_wq(r):
                st = states.setdefault(r, {})
                wq_sb = wpool.tile([128, 8, DL], F8E3, tag="w8", bufs=3,
                                   name=f"r{r}_w_q")
                nc.sync.dma_start(
                    out=wq_sb[:],
                    in_=wqt[:].rearrange("(kc p) m -> p kc m", p=128))
                st["wq"] = wq_sb

            def emit_xq(r):
                st = states.setdefault(r, {})
                xq_l = []
                for kc in range(8):
                    xt = xp.tile([128, S], F8E3, tag="x8", bufs=16,
                                 name=f"r{r}_xq_{kc}")
                    nc.sync.dma_start(out=xt[:],
                                      in_=xq[kc * 128:(kc + 1) * 128, :])
                    xq_l.append(xt)
                st["xq"] = xq_l

            def emit_wo(r):
                st = states.setdefault(r, {})
                wo_sb = wopool.tile([128, 4, D], BF16, tag="wo", bufs=2,
                                    name=f"r{r}_wo_sb")
                nc.sync.dma_start(
                    out=wo_sb[:],
                    in_=wot[:].rearrange("(t p) n -> p t n", p=128))
                st["wo"] = wo_sb

            def alloc_kqa(r):
                st = states.setdefault(r, {})
                st["kh"] = [
                    kqa.tile([128, S], F8E3, tag="kq", bufs=12,
                             name=f"r{r}_khT_{t}") for t in range(4)
                ]
                st["qh"] = [
                    kqa.tile([128, S], F8E3, tag="kq", bufs=12,
                             name=f"r{r}_qhT_{t}") for t in range(4)
                ]
                st["attn"] = [
                    kqa.tile([128, S], BF16, tag="attn", bufs=5,
                             name=f"r{r}_attn_{t}") for t in range(4)
                ]

            # ---------------- PE work units ----------------
            def v_half(r, sti, half):
                # half of one [128(seq), 512(dl)] block of the V projection
                st = states[r]
                if half == 0:
                    acc = prp.tile([128, 512], F32, tag="pr",
                                   name=f"r{r}_psv_{sti}")
                    proj_accs[(r, "v", sti)] = acc
                else:
                    acc = proj_accs.pop((r, "v", sti))
                for kc in range(4 * half, 4 * half + 4):
                    nc.tensor.matmul(
                        acc[:],
                        st["xv"][kc][:, sti * 128:(sti + 1) * 128],
                        st["wv"][:, kc, :],
                        start=(kc == 0),
                        stop=(kc == 7),
                    )
                if half == 1:
                    vt = vhp.tile([128, NH, DH + 1], BF16, tag="vh",
                                  name=f"r{r}_vh_{sti}")
                    nc.vector.tensor_copy(
                        vt[:, :, 0:DH],
                        acc[:].rearrange("p (h d) -> p h d", d=DH))
                    nc.vector.memset(vt[:, :, DH:DH + 1], 1.0)
                    st["vh"][sti] = vt

            def proj_half(r, name, mc, nt, half):
                # half of one [128(dl), 512(seq)] block of K or Q proj; the
                # PSUM accumulator persists between the two halves
                st = states[r]
                w_sb = st["wk"] if name == "k" else st["wq"]
                x_sb = st["xk"] if name == "k" else st["xq"]
                dest = (st["kh"] if name == "k" else st["qh"])[mc]
                if half == 0:
                    acc = prp.tile([128, 512], F32, tag="pr",
                                   name=f"r{r}_ps{name}_{mc}_{nt}")
                    proj_accs[(r, name, mc, nt)] = acc
                else:
                    acc = proj_accs.pop((r, name, mc, nt))
                for kc in range(4 * half, 4 * half + 4):
                    nc.tensor.matmul(
                        acc[:],
                        w_sb[:, kc, mc * 128:(mc + 1) * 128],
                        x_sb[kc][:, nt * 512:(nt + 1) * 512],
                        start=(kc == 0),
                        stop=(kc == 7),
                    )
                if half == 1:
                    # kh/qh stored fp8e3m4 scaled by 2^-6 (psum carries
                    # x*2 and w*64 host scales -> qh_store = qh_raw*2)
                    nc.vector.tensor_scalar(
                        dest[:, nt * 512:(nt + 1) * 512], acc[:],
                        2.0 ** -6, None, mybir.AluOpType.mult)

            def outproj_half(r, qb, nt, half):
                st = states[r]
                if half == 0:
                    acc = prp.tile([128, 512], F32, tag="pr",
                                   name=f"r{r}_psy_{qb}_{nt}")
                    proj_accs[(r, "y", qb, nt)] = acc
                else:
                    acc = proj_accs.pop((r, "y", qb, nt))
                for t in (2 * half, 2 * half + 1):
                    nc.tensor.matmul(
                        acc[:],
                        st["attn"][t][:, qb * 128:(qb + 1) * 128],
                        st["wo"][:, t, nt * 512:(nt + 1) * 512],
                        start=(t == 0),
                        stop=(t == 3),
                    )
                if half == 0:
                    return
                stg = stgp.tile([128, 512], F32, tag="ystg",
                                name=f"r{r}_sty_{qb}_{nt}")
                nc.vector.tensor_copy(stg[:], acc[:])
                nc.sync.dma_start(
                    out=ypart[qb * 128:(qb + 1) * 128,
                              nt * 512:(nt + 1) * 512],
                    in_=stg[:],
                )
                if nt == 1 and qb % 2 == 1:
                    ch = qb // 2
                    if collective:
                        nc.gpsimd.collective_compute(
                            "ReduceScatter",
                            mybir.AluOpType.add,
                            replica_groups=[[0, 1], [2, 3], [4, 5], [6, 7]],
                            ins=[ypart[256 * ch:256 * (ch + 1), :].opt()],
                            outs=[yrs[128 * ch:128 * (ch + 1), :].opt()],
                        )
                        nc.sync.dma_start(
                            out=y[128 * ch:128 * (ch + 1), :],
                            in_=yrs[128 * ch:128 * (ch + 1), :],
                        )
                    elif ch < 4:
                        nc.sync.dma_start(
                            out=y[256 * ch:256 * (ch + 1), :],
                            in_=ypart[256 * ch:256 * (ch + 1), :],
                        )

            # ---------------- filler queue machinery ----------------
            def queue(key, cost, fn, args):
                fillers[key] = (cost, fn, args)

            def emit_one():
                if not fillers:
                    return False
                _, (cost, fn, args) = fillers.popitem(last=False)
                budget[0] = max(budget[0] - cost, -900.0)
                fn(*args)
                return True

            def pop_budget(add):
                budget[0] = min(budget[0] + add, 2200.0)
                while fillers:
                    _, (cost, _, _) = next(iter(fillers.items()))
                    if budget[0] < cost:
                        break
                    emit_one()

            def ensure_filler(key):
                # emit the unit containing `key` now (both halves, in
                # order), jumping the FIFO; other queued units stay put.
                if key not in fillers:
                    return
                unit = key[:-1]
                for h in (0, 1):
                    k2 = unit + (h,)
                    if k2 in fillers:
                        cost, fn, args = fillers.pop(k2)
                        budget[0] = max(budget[0] - cost, -900.0)
                        fn(*args)

            def queue_v(r, lo, hi):
                for sti in range(lo, hi):
                    for h in range(2):
                        queue((r, "v", sti, h), COST_PROJ, v_half,
                              (r, sti, h))

            def queue_proj(r, name, t, nts=range(4)):
                for nt in nts:
                    for h in range(2):
                        queue((r, name, t, nt, h), COST_PROJ, proj_half,
                              (r, name, t, nt, h))

            # ---------------- softmax-normalize tail ----------------
            def make_tail(r, t, qc, pv, ex15):
                # last pv accumulation + softmax normalize of a chain,
                # deferred into the next chain's first kb so the next
                # chain's first scores matmul (and its ACT) aren't
                # queued behind it on the in-order PE
                qsl = slice(qc * W, (qc + 1) * W)
                st = states[r]

                def tail():
                    ensure_filler((r, "v", 15, 1))
                    for p in range(2):
                        nc.tensor.matmul(
                            pv[p][:],
                            st["vh"][15][:, 2 * t + p, :],
                            ex15[:, p * W:(p + 1) * W],
                            start=False,
                            stop=True,
                        )
                    for p in range(2):
                        pvs = pvsp.tile([DH + 1, W], F32, tag="pvs",
                                        name=f"r{r}_pvs_{t}_{qc}_{p}")
                        nc.vector.tensor_copy(pvs[:], pv[p][:])
                        rc = rcp.tile([1, W], F32, tag="rc",
                                      name=f"r{r}_rc_{t}_{qc}_{p}")
                        nc.vector.reciprocal(rc[:], pvs[DH:DH + 1, :])
                        rb = rbp.tile([DH, W], F32, tag="rb",
                                      name=f"r{r}_rb_{t}_{qc}_{p}")
                        nc.gpsimd.partition_broadcast(rb[:], rc[:])
                        nc.vector.tensor_mul(
                            st["attn"][t][64 * p:64 * p + 64, qsl],
                            pvs[0:DH, :], rb[:]
                        )
                return tail

            # ---------------- main loop ----------------
            prev_tail = None
            pending = []   # outproj chunks spilled to the next qc / rep

            for rep in range(repeat):
                last = rep + 1 >= repeat
                nrep = rep + 1

                if rep == 0:
                    # serial prelude: inputs + V proj + K/Q pair-0 lead-in
                    emit_v_inputs(0)
                    for sti in range(16):
                        v_half(0, sti, 0)
                        v_half(0, sti, 1)
                    emit_k_inputs(0)
                    emit_wq(0)
                    emit_xq(0)
                    emit_wo(0)
                    alloc_kqa(0)
                    for nt in range(4):
                        proj_half(0, "k", 0, nt, 0)
                        proj_half(0, "k", 0, nt, 1)
                    proj_half(0, "q", 0, 0, 0)
                    proj_half(0, "q", 0, 0, 1)

                st_r = states[rep]
                q_emitted = {(t, 0) for t in range(4)}

                for qc in range(NQC):
                    # outproj chunks spilled from the previous qc (or the
                    # previous rep's qc3) go first so their attn/wo slot
                    # reads retire early; chain-start ensures jump the FIFO
                    for key, cost, fn, args in pending:
                        queue(key, cost, fn, args)
                    pending = []
                    if rep > 0 or qc > 0:
                        pass
                    if qc == 0 and True:
                        # remaining K/Q lead work for this rep (pair 0 was
                        # handled by the previous rep / the rep-0 prelude)
                        for t in range(1, 4):
                            queue_proj(rep, "k", t)
                            queue_proj(rep, "q", t, nts=(0,))
                    for t in range(4):
                        if (t, qc) not in q_emitted:
                            q_emitted.add((t, qc))
                            queue_proj(rep, "q", t, nts=(qc,))
                    if not last:
                        if qc == 1:
                            emit_v_inputs(nrep)
                            emit_k_inputs(nrep)
                            queue_v(nrep, 0, 8)
                        elif qc == 2:
                            alloc_kqa(nrep)
                            queue_v(nrep, 8, 16)
                        elif qc == 3:
                            emit_wq(nrep)
                            queue_proj(nrep, "k", 0)

                    for t in range(4):
                        # inputs this chain depends on must be emitted first
                        for nt in range(4):
                            ensure_filler((rep, "k", t, nt, 1))
                        ensure_filler((rep, "q", t, qc, 1))
                        if qc == 3 and t == 3 and not last:
                            # all Q chunks of this rep are now emitted, so
                            # the next rep's xq can reuse their ring slots
                            emit_xq(nrep)
                            emit_wo(nrep)
                            queue_proj(nrep, "q", 0, nts=(0,))
                        kh = st_r["kh"][t]
                        qh = st_r["qh"][t]
                        qsl = slice(qc * W, (qc + 1) * W)
                        pv = None
                        ex_tiles = [None] * 16
                        for kb in range(16):
                            sc = scp.tile([128, 2 * W], F32, tag="sc",
                                          name=f"r{rep}_sc_{t}_{qc}_{kb}")
                            ksl = slice(kb * 128, (kb + 1) * 128)
                            # two concurrent row-tiled matmuls (tiles
                            # T0/T8), each writing its own PSUM bank of sc
                            for p in range(2):
                                hsl = slice(64 * p, 64 * p + 64)
                                nc.tensor.matmul(
                                    sc[:, p * W:(p + 1) * W],
                                    kh[hsl, ksl],
                                    qh[hsl, qsl],
                                    start=True,
                                    stop=True,
                                )
                            ex = expp.tile([128, 2 * W], BF16, tag="exp",
                                           name=f"r{rep}_ex_{t}_{qc}_{kb}")
                            ex_tiles[kb] = ex
                            nc.scalar.activation(
                                ex[:], sc[:],
                                mybir.ActivationFunctionType.Exp,
                                scale=SCALE / 4.0,
                            )
                            if kb == 0 and prev_tail is not None:
                                # previous chain's last pv + normalize land
                                # here, after this chain's first ACT
                                prev_tail()
                                prev_tail = None
                            # pv accumulation for previous kb emitted after
                            # this kb's scores to keep ACT double-buffered
                            if kb > 0:
                                if pv is None:
                                    # allocated after the previous chain's
                                    # normalize is emitted so pool-slot
                                    # reuse dependencies are correct
                                    pv = [
                                        pvp.tile([DH + 1, W], F32, tag="pv",
                                                 name=f"r{rep}_pv_{t}_{qc}_{p}")
                                        for p in range(2)
                                    ]
                                ensure_filler((rep, "v", kb - 1, 1))
                                for p in range(2):
                                    nc.tensor.matmul(
                                        pv[p][:],
                                        st_r["vh"][kb - 1][:, 2 * t + p, :],
                                        ex_tiles[kb - 1][:, p * W:(p + 1) * W],
                                        start=(kb - 1 == 0),
                                        stop=False,
                                    )
                                pop_budget(filler_ns)
                        prev_tail = make_tail(rep, t, qc, pv, ex_tiles[15])

                    # output projection for this qc's 4 row-blocks
                    if "c" in phases:
                        chunks = [
                            ((rep, "y", qb, nt, h), COST_OUT,
                             outproj_half, (rep, qb, nt, h))
                            for qb in range(4 * qc, 4 * qc + 4)
                            for nt in range(2)
                            for h in range(2)
                        ]
                        if qc < NQC - 1 or not last:
                            pending = chunks
                        else:
                            # final rep: drain everything serially
                            prev_tail()
                            prev_tail = None
                            while emit_one():
                                pass
                            for qb in range(4 * qc, 4 * qc + 4):
                                for nt in range(2):
                                    outproj_half(rep, qb, nt, 0)
                                    outproj_half(rep, qb, nt, 1)

    nc.finalize()
    return nc


def _get_nc():
    global _NC_CACHE
    if _NC_CACHE is None:
        _NC_CACHE = _build_nc()
    return _NC_CACHE


def make_in_maps(q, k, v, wq, wk, wv, wo):
    bf = ml_dtypes.bfloat16
    f8 = ml_dtypes.float8_e3m4
    in_maps = []
    for c in range(N_CORES):
        b, g = c // 2, c % 2
        sl = slice(DL * g, DL * (g + 1))
        in_maps.append({
            "xq": np.ascontiguousarray(q[b].T * 2.0).astype(f8),
            "xk": np.ascontiguousarray(k[b].T * 2.0).astype(f8),
            "xv": np.ascontiguousarray(v[b].T).astype(bf),
            "wqt": np.ascontiguousarray(wq[sl, :].T * 64.0).astype(f8),
            "wkt": np.ascontiguousarray(wk[sl, :].T * 64.0).astype(f8),
            "wvt": np.ascontiguousarray(wv[sl, :].T).astype(bf),
            "wot": np.ascontiguousarray(wo[:, sl].T).astype(bf),
        })
    return in_maps


def kernel(q, k, v, wq, wk, wv, wo, _res_hook=None):
    q = np.asarray(q, dtype=np.float32)
    k = np.asarray(k, dtype=np.float32)
    v = np.asarray(v, dtype=np.float32)
    wq = np.asarray(wq, dtype=np.float32)
    wk = np.asarray(wk, dtype=np.float32)
    wv = np.asarray(wv, dtype=np.float32)
    wo = np.asarray(wo, dtype=np.float32)
    B = q.shape[0]

    nc = _get_nc()
    in_maps = make_in_maps(q, k, v, wq, wk, wv, wo)

    res = run_bass_kernel_spmd(nc, in_maps, list(range(N_CORES)))
    if _res_hook is not None:
        _res_hook(res)

    out = np.empty((B, S, D), dtype=np.float32)
    for c in range(N_CORES):
        b, g = c // 2, c % 2
        yc = res.results[c]["y"]
        for ch in range(8):
            out[b, 256 * ch + 128 * g:256 * ch + 128 * (g + 1), :] = \
                yc[128 * ch:128 * (ch + 1), :]
    return out


# revision 11
# speedup vs baseline: 1.0449x; 1.0449x over previous
"""Multi-head attention (B=4, S=2048, D=1024, H=16) on 8 TRN2 NeuronCores.

Sharding: core c = 2*b + g handles batch b (of 4) and head-group g (of 2,
8 heads / 512 model dims each).  Per core:
  - QKV projections for its batch restricted to its 512 output dims;
    qhT/khT [512, 2048] (fp8e3m4, scaled) and vh [2048, 520] (bf16) stay
    resident in SBUF.  Q/K x-inputs and weights arrive as scaled fp8e3m4
    (x*2, w*64; the 4x on scores is folded into the exp scale).
  - attention for its 8 heads in transposed-scores layout (scoresT[k, q]):
    softmax denominator via a ones-column appended to V; no max subtraction
    (scores are ~N(0, 0.08^2) after the 1/32 scale, exp cannot overflow)
  - structured to keep ScalarE (exp) saturated: per (head-pair, 512-wide
    q chunk) the two heads' scores land in one [128, 1024] PSUM tile
    (different banks, written by two concurrent 64x128 row-tiled matmuls)
    so one ACT instruction covers both heads; sc double-buffered.
  - ALL projection work (V/K/Q of the NEXT iteration, output projection of
    the PREVIOUS qc / iteration) is emitted as cost-budgeted "filler"
    chunks inside the attention kb loop, so ScalarE never idles across
    iteration boundaries: iteration r+1's V proj fillers run during r's
    qc 1-2, its K/Q pair-0 during r's qc3, and r's last outproj chunks
    during r+1's qc0.
  - output projection partial over its 512 model dims; partials
    ReduceScatter'd pairwise in 8 chunks so the collective overlaps compute
Host: pre-transposes inputs/weights, feeds per-core shards, and
reassembles the full [4, 2048, 1024] fp32 output from the 8 per-core
outputs (chunked-RS row interleaving: core 2b+g holds rows
256*ch + [128*g, 128*(g+1)) of batch b for ch in 0..7).
"""

from collections import OrderedDict

import numpy as np
import ml_dtypes

import concourse.bass as bass
import concourse.mybir as mybir
import concourse.tile as tile
from concourse import bacc
from concourse.bass_utils import run_bass_kernel_spmd

N_CORES = 8
S = 2048          # sequence length
D = 1024          # d_model
DL = 512          # local model dims (8 heads x 64)
NH = 8            # local heads
DH = 64           # head dim
W = 512           # q-chunk width for attention
NQC = S // W      # 4 q chunks
SCALE = 1.0 / 32.0  # 1/sqrt(d_model)

F32 = mybir.dt.float32
BF16 = mybir.dt.bfloat16
F8E3 = mybir.dt.float8e3

COST_PROJ = 860.0   # ns: 4-matmul proj/V half-chunk
COST_OUT = 430.0    # ns: 2-matmul outproj half-chunk

_NC_CACHE = None


def _build_nc(repeat=1, phases="abc", collective=True, overlap_c=True,
              filler_ns=540.0):
    nc = bacc.Bacc("TRN2", target_bir_lowering=False, debug=False,
                   num_devices=N_CORES)

    xq = nc.dram_tensor("xq", [D, S], F8E3, kind="ExternalInput")
    xk = nc.dram_tensor("xk", [D, S], F8E3, kind="ExternalInput")
    xv = nc.dram_tensor("xv", [D, S], BF16, kind="ExternalInput")
    wqt = nc.dram_tensor("wqt", [D, DL], F8E3, kind="ExternalInput")
    wkt = nc.dram_tensor("wkt", [D, DL], F8E3, kind="ExternalInput")
    wvt = nc.dram_tensor("wvt", [D, DL], BF16, kind="ExternalInput")
    wot = nc.dram_tensor("wot", [DL, D], BF16, kind="ExternalInput")
    y = nc.dram_tensor("y", [S // 2, D], F32, kind="ExternalOutput")

    ypart = nc.dram_tensor("ypart", [S, D], F32)
    yrs = nc.dram_tensor("yrs", [S // 2, D], F32)

    with tile.TileContext(nc) as tc:
        with (
            tc.tile_pool(name="xp", bufs=9) as xp,           # x input chunks
            tc.tile_pool(name="kqa", bufs=12) as kqa,        # khT/qhT/attn
            tc.tile_pool(name="wp", bufs=1) as wpool,        # wq/wk/wv
            tc.tile_pool(name="wop", bufs=2) as wopool,      # woT
            tc.tile_pool(name="vhp", bufs=32) as vhp,        # vh | ones
            tc.tile_pool(name="expp", bufs=5) as expp,       # exp(scores)
            tc.tile_pool(name="pvsp", bufs=2) as pvsp,       # pv psum drain
            tc.tile_pool(name="rcp", bufs=2) as rcp,         # reciprocal row
            tc.tile_pool(name="rbp", bufs=2) as rbp,         # bcast recip
            tc.tile_pool(name="stgp", bufs=2) as stgp,       # psum->dram stg
            tc.tile_pool(name="scp", bufs=2, space="PSUM") as scp,   # 4 bank
            tc.tile_pool(name="pvp", bufs=2, space="PSUM") as pvp,   # 2 bank
            tc.tile_pool(name="prp", bufs=2, space="PSUM") as prp,   # 2 bank
        ):
            fillers = OrderedDict()   # key -> (cost_ns, fn, args), FIFO
            states = {}               # rep -> tiles dict
            proj_accs = {}            # (rep, name, a[, b]) -> psum acc
            budget = [0.0]

            # ---------------- input emitters (DMA only) ----------------
            def emit_v_inputs(r):
                st = states.setdefault(r, {})
                wv_sb = wpool.tile([128, 8, DL], BF16, tag="w",
                                   name=f"r{r}_w_v")
                nc.sync.dma_start(
                    out=wv_sb[:],
                    in_=wvt[:].rearrange("(kc p) m -> p kc m", p=128))
                xv_l = []
                for kc in range(8):
                    xt = xp.tile([128, S], BF16, tag="x", bufs=9,
                                 name=f"r{r}_xv_{kc}")
                    nc.sync.dma_start(out=xt[:],
                                      in_=xv[kc * 128:(kc + 1) * 128, :])
                    xv_l.append(xt)
                st["wv"], st["xv"] = wv_sb, xv_l
                st["vh"] = [None] * 16

            def emit_k_inputs(r):
                st = states.setdefault(r, {})
                wk_sb = wpool.tile([128, 8, DL], F8E3, tag="w8", bufs=3,
                                   name=f"r{r}_w_k")
                nc.sync.dma_start(
                    out=wk_sb[:],
                    in_=wkt[:].rearrange("(kc p) m -> p kc m", p=128))
                xk_l = []
                for kc in range(8):
                    xt = xp.tile([128, S], F8E3, tag="x8", bufs=16,
                                 name=f"r{r}_xk_{kc}")
                    nc.sync.dma_start(out=xt[:],
                                      in_=xk[kc * 128:(kc + 1) * 128, :])
                    xk_l.append(xt)
                st["wk"], st["xk"] = wk_sb, xk_l

            def emit_wq(r):
                st = states.setdefault(r, {})
                wq_sb = wpool.tile([128, 8, DL], F8E3, tag="w8", bufs=3,
                                   name=f"r{r}_w_q")
                nc.sync.dma_start(
                    out=wq_sb[:],
                    in_=wqt[:].rearrange("(kc p) m -> p kc m", p=128))
                st["wq"] = wq_sb

            def emit_xq(r):
                st = states.setdefault(r, {})
                xq_l = []
                for kc in range(8):
                    xt = xp.tile([128, S], F8E3, tag="x8", bufs=16,
                                 name=f"r{r}_xq_{kc}")
                    nc.sync.dma_start(out=xt[:],
                                      in_=xq[kc * 128:(kc + 1) * 128, :])
                    xq_l.append(xt)
                st["xq"] = xq_l

            def emit_wo(r):
                st = states.setdefault(r, {})
                wo_sb = wopool.tile([128, 4, D], BF16, tag="wo", bufs=2,
                                    name=f"r{r}_wo_sb")
                nc.sync.dma_start(
                    out=wo_sb[:],
                    in_=wot[:].rearrange("(t p) n -> p t n", p=128))
                st["wo"] = wo_sb

            def alloc_kqa(r):
                st = states.setdefault(r, {})
                st["kh"] = [
                    kqa.tile([128, S], F8E3, tag="kq", bufs=12,
                             name=f"r{r}_khT_{t}") for t in range(4)
                ]
                st["qh"] = [
                    kqa.tile([128, S], F8E3, tag="kq", bufs=12,
                             name=f"r{r}_qhT_{t}") for t in range(4)
                ]
                st["attn"] = [
                    kqa.tile([128, S], BF16, tag="attn", bufs=5,
                             name=f"r{r}_attn_{t}") for t in range(4)
                ]

            # ---------------- PE work units ----------------
            def v_half(r, sti, half):
                # half of one [128(seq), 512(dl)] block of the V projection
                st = states[r]
                if half == 0:
                    acc = prp.tile([128, 512], F32, tag="pr",
                                   name=f"r{r}_psv_{sti}")
                    proj_accs[(r, "v", sti)] = acc
                else:
                    acc = proj_accs.pop((r, "v", sti))
                for kc in range(4 * half, 4 * half + 4):
                    nc.tensor.matmul(
                        acc[:],
                        st["xv"][kc][:, sti * 128:(sti + 1) * 128],
                        st["wv"][:, kc, :],
                        start=(kc == 0),
                        stop=(kc == 7),
                    )
                if half == 1:
                    vt = vhp.tile([128, NH, DH + 1], BF16, tag="vh",
                                  name=f"r{r}_vh_{sti}")
                    nc.vector.tensor_copy(
                        vt[:, :, 0:DH],
                        acc[:].rearrange("p (h d) -> p h d", d=DH))
                    nc.vector.memset(vt[:, :, DH:DH + 1], 1.0)
                    st["vh"][sti] = vt

            def proj_half(r, name, mc, nt, half):
                # half of one [128(dl), 512(seq)] block of K or Q proj; the
                # PSUM accumulator persists between the two halves
                st = states[r]
                w_sb = st["wk"] if name == "k" else st["wq"]
                x_sb = st["xk"] if name == "k" else st["xq"]
                dest = (st["kh"] if name == "k" else st["qh"])[mc]
                if half == 0:
                    acc = prp.tile([128, 512], F32, tag="pr",
                                   name=f"r{r}_ps{name}_{mc}_{nt}")
                    proj_accs[(r, name, mc, nt)] = acc
                else:
                    acc = proj_accs.pop((r, name, mc, nt))
                for kc in range(4 * half, 4 * half + 4):
                    nc.tensor.matmul(
                        acc[:],
                        w_sb[:, kc, mc * 128:(mc + 1) * 128],
                        x_sb[kc][:, nt * 512:(nt + 1) * 512],
                        start=(kc == 0),
                        stop=(kc == 7),
                    )
                if half == 1:
                    # kh/qh stored fp8e3m4 scaled by 2^-6 (psum carries
                    # x*2 and w*64 host scales -> qh_store = qh_raw*2)
                    nc.vector.tensor_scalar(
                        dest[:, nt * 512:(nt + 1) * 512], acc[:],
                        2.0 ** -6, None, mybir.AluOpType.mult)

            def outproj_half(r, qb, nt, half):
                st = states[r]
                if half == 0:
                    acc = prp.tile([128, 512], F32, tag="pr",
                                   name=f"r{r}_psy_{qb}_{nt}")
                    proj_accs[(r, "y", qb, nt)] = acc
                else:
                    acc = proj_accs.pop((r, "y", qb, nt))
                for t in (2 * half, 2 * half + 1):
                    nc.tensor.matmul(
                        acc[:],
                        st["attn"][t][:, qb * 128:(qb + 1) * 128],
                        st["wo"][:, t, nt * 512:(nt + 1) * 512],
                        start=(t == 0),
                        stop=(t == 3),
                    )
                if half == 0:
                    return
                stg = stgp.tile([128, 512], F32, tag="ystg",
                                name=f"r{r}_sty_{qb}_{nt}")
                nc.vector.tensor_copy(stg[:], acc[:])
                nc.sync.dma_start(
                    out=ypart[qb * 128:(qb + 1) * 128,
                              nt * 512:(nt + 1) * 512],
                    in_=stg[:],
                )
                if nt == 1 and qb % 2 == 1:
                    ch = qb // 2
                    if collective:
                        nc.gpsimd.collective_compute(
                            "ReduceScatter",
                            mybir.AluOpType.add,
                            replica_groups=[[0, 1], [2, 3], [4, 5], [6, 7]],
                            ins=[ypart[256 * ch:256 * (ch + 1), :].opt()],
                            outs=[yrs[128 * ch:128 * (ch + 1), :].opt()],
                        )
                        nc.sync.dma_start(
                            out=y[128 * ch:128 * (ch + 1), :],
                            in_=yrs[128 * ch:128 * (ch + 1), :],
                        )
                    elif ch < 4:
                        nc.sync.dma_start(
                            out=y[256 * ch:256 * (ch + 1), :],
                            in_=ypart[256 * ch:256 * (ch + 1), :],
                        )

            # ---------------- filler queue machinery ----------------
            def queue(key, cost, fn, args):
                fillers[key] = (cost, fn, args)

            def emit_one():
                if not fillers:
                    return False
                _, (cost, fn, args) = fillers.popitem(last=False)
                budget[0] = max(budget[0] - cost, -900.0)
                fn(*args)
                return True

            def pop_budget(add):
                budget[0] = min(budget[0] + add, 2200.0)
                while fillers:
                    _, (cost, _, _) = next(iter(fillers.items()))
                    if budget[0] < cost:
                        break
                    emit_one()

            def ensure_filler(key):
                # emit the unit containing `key` now (both halves, in
                # order), jumping the FIFO; other queued units stay put.
                if key not in fillers:
                    return
                unit = key[:-1]
                for h in (0, 1):
                    k2 = unit + (h,)
                    if k2 in fillers:
                        cost, fn, args = fillers.pop(k2)
                        budget[0] = max(budget[0] - cost, -900.0)
                        fn(*args)

            def queue_v(r, lo, hi):
                for sti in range(lo, hi):
                    for h in range(2):
                        queue((r, "v", sti, h), COST_PROJ, v_half,
                              (r, sti, h))

            def queue_proj(r, name, t, nts=range(4)):
                for nt in nts:
                    for h in range(2):
                        queue((r, name, t, nt, h), COST_PROJ, proj_half,
                              (r, name, t, nt, h))

            # ---------------- softmax-normalize tail ----------------
            def make_tail(r, t, qc, pv, ex15):
                # last pv accumulation + softmax normalize of a chain,
                # deferred into the next chain's first kb so the next
                # chain's first scores matmul (and its ACT) aren't
                # queued behind it on the in-order PE
                qsl = slice(qc * W, (qc + 1) * W)
                st = states[r]

                def tail():
                    ensure_filler((r, "v", 15, 1))
                    for p in range(2):
                        nc.tensor.matmul(
                            pv[p][:],
                            st["vh"][15][:, 2 * t + p, :],
                            ex15[:, p * W:(p + 1) * W],
                            start=False,
                            stop=True,
                        )
                    for p in range(2):
                        pvs = pvsp.tile([DH + 1, W], F32, tag="pvs",
                                        name=f"r{r}_pvs_{t}_{qc}_{p}")
                        nc.vector.tensor_copy(pvs[:], pv[p][:])
                        rc = rcp.tile([1, W], F32, tag="rc",
                                      name=f"r{r}_rc_{t}_{qc}_{p}")
                        nc.vector.reciprocal(rc[:], pvs[DH:DH + 1, :])
                        rb = rbp.tile([DH, W], F32, tag="rb",
                                      name=f"r{r}_rb_{t}_{qc}_{p}")
                        nc.gpsimd.partition_broadcast(rb[:], rc[:])
                        nc.vector.tensor_mul(
                            st["attn"][t][64 * p:64 * p + 64, qsl],
                            pvs[0:DH, :], rb[:]
                        )
                return tail

            # ---------------- main loop ----------------
            prev_tail = None
            pending = []   # outproj chunks spilled to the next qc / rep

            for rep in range(repeat):
                last = rep + 1 >= repeat
                nrep = rep + 1

                if rep == 0:
                    # serial prelude: inputs + V proj + K/Q pair-0 lead-in
                    emit_v_inputs(0)
                    for sti in range(16):
                        v_half(0, sti, 0)
                        v_half(0, sti, 1)
                    emit_k_inputs(0)
                    emit_wq(0)
                    emit_xq(0)
                    emit_wo(0)
                    alloc_kqa(0)
                    for nt in range(4):
                        proj_half(0, "k", 0, nt, 0)
                        proj_half(0, "k", 0, nt, 1)
                    proj_half(0, "q", 0, 0, 0)
                    proj_half(0, "q", 0, 0, 1)

                st_r = states[rep]
                q_emitted = {(t, 0) for t in range(4)}

                for qc in range(NQC):
                    # outproj chunks spilled from the previous qc (or the
                    # previous rep's qc3) go first so their attn/wo slot
                    # reads retire early; chain-start ensures jump the FIFO
                    take = pending if len(pending) <= 16 else pending[:16]
                    rest = [] if len(pending) <= 16 else pending[16:]
                    for key, cost, fn, args in take:
                        queue(key, cost, fn, args)
                    pending = rest
                    if rep > 0 or qc > 0:
                        pass
                    if qc == 0:
                        # remaining K lead work for this rep (pair 0 and all
                        # Q(t,0) were queued by the previous rep; rep 0
                        # queues its own Q(t,0) here)
                        for t in range(1, 4):
                            queue_proj(rep, "k", t)
                            if rep == 0:
                                queue_proj(rep, "q", t, nts=(0,))
                    for t in range(4):
                        if (t, qc) not in q_emitted:
                            q_emitted.add((t, qc))
                            queue_proj(rep, "q", t, nts=(qc,))
                    if not last:
                        if qc == 1:
                            emit_v_inputs(nrep)
                            emit_k_inputs(nrep)
                            queue_v(nrep, 0, 8)
                        elif qc == 2:
                            alloc_kqa(nrep)
                            queue_v(nrep, 8, 16)
                        elif qc == 3:
                            emit_wq(nrep)
                            queue_proj(nrep, "k", 0)

                    for t in range(4):
                        # inputs this chain depends on must be emitted first
                        for nt in range(4):
                            ensure_filler((rep, "k", t, nt, 1))
                        ensure_filler((rep, "q", t, qc, 1))
                        if qc == 3 and t == 3 and not last:
                            # all Q chunks of this rep are now emitted, so
                            # the next rep's xq can reuse their ring slots
                            emit_xq(nrep)
                            emit_wo(nrep)
                            for tq in range(4):
                                queue_proj(nrep, "q", tq, nts=(0,))
                        kh = st_r["kh"][t]
                        qh = st_r["qh"][t]
                        qsl = slice(qc * W, (qc + 1) * W)
                        pv = None
                        ex_tiles = [None] * 16
                        for kb in range(16):
                            sc = scp.tile([128, 2 * W], F32, tag="sc",
                                          name=f"r{rep}_sc_{t}_{qc}_{kb}")
                            ksl = slice(kb * 128, (kb + 1) * 128)
                            # two concurrent row-tiled matmuls (tiles
                            # T0/T8), each writing its own PSUM bank of sc
                            for p in range(2):
                                hsl = slice(64 * p, 64 * p + 64)
                                nc.tensor.matmul(
                                    sc[:, p * W:(p + 1) * W],
                                    kh[hsl, ksl],
                                    qh[hsl, qsl],
                                    start=True,
                                    stop=True,
                                )
                            ex = expp.tile([128, 2 * W], BF16, tag="exp",
                                           name=f"r{rep}_ex_{t}_{qc}_{kb}")
                            ex_tiles[kb] = ex
                            nc.scalar.activation(
                                ex[:], sc[:],
                                mybir.ActivationFunctionType.Exp,
                                scale=SCALE / 4.0,
                            )
                            if kb == 0 and prev_tail is not None:
                                # previous chain's last pv + normalize land
                                # here, after this chain's first ACT
                                prev_tail()
                                prev_tail = None
                            # pv accumulation for previous kb emitted after
                            # this kb's scores to keep ACT double-buffered
                            if kb > 0:
                                if pv is None:
                                    # allocated after the previous chain's
                                    # normalize is emitted so pool-slot
                                    # reuse dependencies are correct
                                    pv = [
                                        pvp.tile([DH + 1, W], F32, tag="pv",
                                                 name=f"r{rep}_pv_{t}_{qc}_{p}")
                                        for p in range(2)
                                    ]
                                ensure_filler((rep, "v", kb - 1, 1))
                                for p in range(2):
                                    nc.tensor.matmul(
                                        pv[p][:],
                                        st_r["vh"][kb - 1][:, 2 * t + p, :],
                                        ex_tiles[kb - 1][:, p * W:(p + 1) * W],
                                        start=(kb - 1 == 0),
                                        stop=False,
                                    )
                            pop_budget(filler_ns)
                        prev_tail = make_tail(rep, t, qc, pv, ex_tiles[15])

                    # output projection for this qc's 4 row-blocks
                    if "c" in phases:
                        chunks = [
                            ((rep, "y", qb, nt, h), COST_OUT,
                             outproj_half, (rep, qb, nt, h))
                            for qb in range(4 * qc, 4 * qc + 4)
                            for nt in range(2)
                            for h in range(2)
                        ]
                        if qc < NQC - 1 or not last:
                            pending = chunks
                        else:
                            # final rep: drain everything serially
                            prev_tail()
                            prev_tail = None
                            while emit_one():
                                pass
                            for qb in range(4 * qc, 4 * qc + 4):
                                for nt in range(2):
                                    outproj_half(rep, qb, nt, 0)
                                    outproj_half(rep, qb, nt, 1)

    nc.finalize()
    return nc


def _get_nc():
    global _NC_CACHE
    if _NC_CACHE is None:
        _NC_CACHE = _build_nc()
    return _NC_CACHE


def make_in_maps(q, k, v, wq, wk, wv, wo):
    bf = ml_dtypes.bfloat16
    f8 = ml_dtypes.float8_e3m4
    in_maps = []
    for c in range(N_CORES):
        b, g = c // 2, c % 2
        sl = slice(DL * g, DL * (g + 1))
        in_maps.append({
            "xq": np.ascontiguousarray(q[b].T * 2.0).astype(f8),
            "xk": np.ascontiguousarray(k[b].T * 2.0).astype(f8),
            "xv": np.ascontiguousarray(v[b].T).astype(bf),
            "wqt": np.ascontiguousarray(wq[sl, :].T * 64.0).astype(f8),
            "wkt": np.ascontiguousarray(wk[sl, :].T * 64.0).astype(f8),
            "wvt": np.ascontiguousarray(wv[sl, :].T).astype(bf),
            "wot": np.ascontiguousarray(wo[:, sl].T).astype(bf),
        })
    return in_maps


def kernel(q, k, v, wq, wk, wv, wo, _res_hook=None):
    q = np.asarray(q, dtype=np.float32)
    k = np.asarray(k, dtype=np.float32)
    v = np.asarray(v, dtype=np.float32)
    wq = np.asarray(wq, dtype=np.float32)
    wk = np.asarray(wk, dtype=np.float32)
    wv = np.asarray(wv, dtype=np.float32)
    wo = np.asarray(wo, dtype=np.float32)
    B = q.shape[0]

    nc = _get_nc()
    in_maps = make_in_maps(q, k, v, wq, wk, wv, wo)

    res = run_bass_kernel_spmd(nc, in_maps, list(range(N_CORES)))
    if _res_hook is not None:
        _res_hook(res)

    out = np.empty((B, S, D), dtype=np.float32)
    for c in range(N_CORES):
        b, g = c // 2, c % 2
        yc = res.results[c]["y"]
        for ch in range(8):
            out[b, 256 * ch + 128 * g:256 * ch + 128 * (g + 1), :] = \
                yc[128 * ch:128 * (ch + 1), :]
    return out
